# revision 30
# baseline (speedup 1.0000x reference)
"""CaNet (moe_routing GNN) forward on 8 Trainium2 NeuronCores.

Sharding: nodes are range-partitioned across the 8 cores (6250 each, padded
to 6272 = 49*128). Each core owns the edges whose *destination* lands in its
node range. The GCN aggregation out[col] += val * h[row] is computed as a
sequence of tiny one-hot matmuls on the TensorEngine:

  - edges are sorted by destination tile (groups of 128 dest nodes) on the
    host and padded to multiples of 128 ("chunks");
  - the source features h[row] for one chunk are fetched from a replicated
    node-major bf16 table in HBM with the GpSimd dma_gather custom op.
    The table is split by *slab half*: rows [0,3200) of every core's padded
    slab form the "lo" table (8*3200=25600 rows), rows [3200,6272) the "hi"
    table (8*3072=24576 rows); both halves are int16-indexable and every
    chunk draws from a single half ("lo"/"hi" streams);
  - lo gathers run on SWDGE queue 0 and hi gathers on queue 1 so one call's
    Q7 descriptor generation overlaps the other call's HBM drain (a shared
    ring otherwise serializes gen -> drain -> gen);
  - a [128e x 128d] selection matrix S (S[e,d] = (d == ldest[e]) * val[e])
    is prebuilt on the host per chunk and streamed from HBM in 32-chunk
    windows (building S on the DVE per chunk serializes the vector engine
    behind gather-gated matmuls);
  - psum_gcnT[f,d] += G_chunk.T @ S accumulates over the tile's chunks.

Dense per-node work (expert gate softmax, the K=4 expert convs, mixing,
residual relu, fc0/fc1) runs in bf16 matmuls + f32 psum per 128-node tile.
Between layers the updated h slab is AllGather'd in two pieces: the first
25 tiles' rows (the lo half) are gathered as soon as they are computed, so
the next layer's lo gather calls start while the hi half is still being
computed/gathered.

The per-core *program* is identical (SPMD); all per-core variation (gather
indices, selection metadata, x slab) arrives via ExternalInputs. Chunk
counts per (tile, half) are max'd across cores so the schedule is static;
padding slots use idx=0 / val=0 and contribute zero. Gather calls are 64
chunks (8192 indices) except a short final call per stream, so there is no
call-level index padding.
"""

import sys

sys.path.insert(0, "/opt/trn_rl_repo")

import numpy as np
import ml_dtypes

import concourse.bacc as bacc
import concourse.tile as tile
import concourse.mybir as mybir
import concourse.bass as bass
from concourse import bass_utils
from concourse.masks import make_identity

# Problem constants (hardcoded per contract).
N = 50000
E = 800000
D = 128  # input dim
H = 128  # hidden dim
C = 47  # classes
K = 4  # experts
L = 2  # conv layers
M = 8  # cores

NPC = N // M  # 6250 nodes per core
T = (NPC + 127) // 128  # 49 tiles per core
NPAD = T * 128  # 6272
TSPL = 25  # tiles in the "lo" slab half
SPL = TSPL * 128  # 3200 rows: slab rows [0,SPL) are "lo", rest "hi"
SPH = NPAD - SPL  # 3072
LO = M * SPL  # 25600 lo-table rows (int16-safe)
HI = M * SPH  # 24576 hi-table rows
CALL = 40  # chunks per full dma_gather call (5120 indices)
SWIN = 32  # chunks per streamed S-matrix window (aligned with gather calls)

F32 = mybir.dt.float32
BF16 = mybir.dt.bfloat16
I16 = mybir.dt.int16
BF = ml_dtypes.bfloat16


def _preprocess(x, edge_index, fc0_w, fc0_b, fc1_w, fc1_b, env_w, env_b, conv_w):
    """Host-side: degree/value computation, edge sort, static chunk schedule,
    per-core gather/selection arrays, weight packing."""
    row = np.asarray(edge_index[0], np.int64)
    col = np.asarray(edge_index[1], np.int64)

    deg = np.bincount(col, minlength=N).astype(np.float32)
    dinv = np.where(deg > 0, 1.0 / np.sqrt(deg), 0.0).astype(np.float32)
    val = (dinv[col] * dinv[row]).astype(np.float32)

    core = col // NPC
    dloc = col % NPC
    tl = dloc // 128
    ld = (dloc % 128).astype(np.float32)
    score = row // NPC  # source core
    soff = row % NPC  # offset within source slab (< NPC <= NPAD)
    half = (soff >= SPL).astype(np.int64)
    # agin buffers are shipped partition-major ([128p, tiles, H]), so table
    # row for node offset off = t*128+p is p*TILES + t within the core block.
    tA = soff // 128
    pA = soff % 128
    tB = (soff - SPL) // 128
    idx16 = np.where(
        half == 1,
        score * SPH + pA * (T - TSPL) + tB,
        score * SPL + pA * TSPL + tA,
    )

    # group = (core, half, tile); count chunks per (tile, half) max'd over cores
    cnt = np.bincount((core * T + tl) * 2 + half, minlength=M * T * 2).reshape(
        M, T, 2
    )
    nch = -(-cnt // 128)  # ceil div, [M, T, 2]
    NCH = nch.max(axis=0)  # [T, 2] static schedule
    assert NCH.sum() > 0
    tot = NCH.sum(axis=0)  # [2] total chunks per stream (no call padding)
    # call sizes: full CALL-chunk calls plus one short final call
    callsz = []
    for s in range(2):
        t_s = int(tot[s])
        cs = [CALL] * (t_s // CALL)
        if t_s % CALL:
            cs.append(t_s % CALL)
        callsz.append(cs)

    # stream-local chunk base per tile (exclusive cumsum), shared across cores
    base = np.zeros((T, 2), np.int64)
    base[1:] = NCH[:-1].cumsum(axis=0)

    # within-group rank for every edge
    gkey = (core * 2 + half) * T + tl
    order = np.argsort(gkey, kind="stable")
    gsort = gkey[order]
    starts = np.searchsorted(gsort, np.arange(M * 2 * T))
    rank = np.arange(E, dtype=np.int64) - starts[gsort]
    # slot within (core, half) stream
    slot = np.empty(E, np.int64)
    slot[order] = base[tl[order], half[order]] * 128 + rank

    idx_arr = np.zeros((M, 2), object)
    ld_arr = np.zeros((M, 2), object)
    v_arr = np.zeros((M, 2), object)
    s_arr = np.zeros((M, 2), object)
    lanes = np.arange(128)
    for c in range(M):
        for s in range(2):
            nslot = int(tot[s]) * 128
            ia = np.zeros(nslot, np.int16)
            la = np.zeros(nslot, np.int64)
            va = np.zeros(nslot, np.float32)
            sel = (core == c) & (half == s)
            ia[slot[sel]] = idx16[sel].astype(np.int16)
            la[slot[sel]] = ld[sel].astype(np.int64)
            va[slot[sel]] = val[sel]
            # wrap indices for dma_gather: per call [16, C*8] tiled x8 -> [128, C*8]
            pieces = []
            off = 0
            for cs in callsz[s]:
                ni = cs * 128
                iw = ia[off : off + ni].reshape(ni // 16, 16).T  # [16, cs*8]
                pieces.append(np.tile(iw, (8, 1)))  # [128, cs*8]
                off += ni
            idx_arr[c, s] = np.concatenate(pieces, axis=1)
            ld_arr[c, s] = la.reshape(int(tot[s]), 128).T.astype(np.float32)
            v_arr[c, s] = va.reshape(int(tot[s]), 128).T.copy()
            # prebuilt selection matrices: S[e, ch, d] = (d == ld)*val
            sf = np.zeros((nslot, 128), BF)
            sf[np.arange(nslot), la] = va.astype(BF)
            sf[va == 0.0] = 0  # padding slots contribute nothing even if ld=0
            s_arr[c, s] = np.ascontiguousarray(
                sf.reshape(int(tot[s]), 128, 128).transpose(1, 0, 2)
            )

    # x slabs, transposed + padded: [128 d, NPAD n] bf16 per core
    x = np.asarray(x, np.float32)
    xT = np.zeros((M, D, NPAD), BF)
    for c in range(M):
        xT[c, :, :NPC] = x[c * NPC : (c + 1) * NPC].T.astype(BF)

    # packed weights
    conv_w = np.asarray(conv_w, np.float32)
    wtop = np.zeros((L, H, K * H), BF)
    wbot = np.zeros((L, H, K * H), BF)
    for l in range(L):
        for k in range(K):
            wtop[l, :, k * H : (k + 1) * H] = conv_w[l, k, :H].astype(BF)
            wbot[l, :, k * H : (k + 1) * H] = conv_w[l, k, H:].astype(BF)
    env_w = np.asarray(env_w, np.float32)
    env_b = np.asarray(env_b, np.float32)
    prep = dict(
        NCH=NCH,
        base=base,
        callsz=callsz,
        tot=tot,
        idx_arr=idx_arr,
        ld_arr=ld_arr,
        v_arr=v_arr,
        s_arr=s_arr,
        xT=xT,
        fc0_w=np.asarray(fc0_w, BF),
        b0=np.asarray(fc0_b, np.float32),
        wtop=wtop,
        wbot=wbot,
        env_w_bf=env_w.astype(BF),
        expb=np.stack([np.tile(np.exp(env_b[l]), (128, 1)) for l in range(L)]),
        fc1_w=np.asarray(fc1_w, BF),
        b1_bcast=np.tile(np.asarray(fc1_b, np.float32), (128, 1)),
    )
    return prep


def _emulate(prep):
    """Numpy mirror of the device program (validates schedule/indexing)."""
    NCH, base, callsz, tot = prep["NCH"], prep["base"], prep["callsz"], prep["tot"]
    h_node = np.zeros((M, NPAD, H), np.float32)
    for c in range(M):
        z = prep["xT"][c].T.astype(np.float32) @ prep["fc0_w"].astype(
            np.float32
        ) + prep["b0"]
        h_node[c] = np.maximum(z, 0.0)

    def tables(hn):
        # p-major within each core: row p*TILES + t = node t*128 + p
        tlo = np.concatenate([
            hn[c, :SPL].reshape(TSPL, 128, H).transpose(1, 0, 2).reshape(SPL, H).astype(BF)
            for c in range(M)
        ])
        thi = np.concatenate([
            hn[c, SPL:].reshape(T - TSPL, 128, H).transpose(1, 0, 2).reshape(SPH, H).astype(BF)
            for c in range(M)
        ])
        return tlo, thi

    tlo, thi = tables(h_node)

    for l in range(L):
        new_h = np.zeros_like(h_node)
        for c in range(M):
            G = [None, None]
            for s in range(2):
                ia = prep["idx_arr"][c, s]
                idxs = []
                off = 0
                for cs in callsz[s]:
                    blkw = ia[:16, off : off + cs * 8]  # [16, cs*8]
                    idxs.append(blkw.T.reshape(-1))
                    off += cs * 8
                idxs = np.concatenate(idxs).astype(np.int64)
                tab = tlo if s == 0 else thi
                G[s] = tab[idxs].astype(np.float32)
            gcnT = np.zeros((T, H, 128), np.float32)
            for t in range(T):
                acc = np.zeros((H, 128), np.float32)
                for s in range(2):
                    for jc in range(NCH[t, s]):
                        ch = base[t, s] + jc
                        g = G[s][ch * 128 : (ch + 1) * 128]
                        S = prep["s_arr"][c, s][:, ch, :].astype(np.float32)
                        acc += g.astype(BF).astype(np.float32).T @ S
                gcnT[t] = acc
            hT_bf = h_node[c].T.astype(BF)
            for t in range(T):
                sl = slice(t * 128, (t + 1) * 128)
                ht = hT_bf[:, sl].astype(np.float32)
                z = ht.T @ prep["env_w_bf"][l].astype(np.float32)
                e = np.exp(z) * prep["expb"][l][0][None, :]
                e = e / e.sum(axis=1, keepdims=True)
                gt = gcnT[t].astype(BF).astype(np.float32)
                O = gt.T @ prep["wtop"][l].astype(np.float32) + ht.T @ prep[
                    "wbot"
                ][l].astype(np.float32)
                O = O.reshape(128, K, H)
                mixed = np.einsum("nk,nkh->nh", e, O)
                new_h[c, sl] = np.maximum(mixed + h_node[c, sl], 0.0)
        h_node = new_h
        tlo, thi = tables(h_node)

    out = np.zeros((N, C), np.float32)
    for c in range(M):
        z = h_node[c] @ prep["fc1_w"].astype(np.float32) + prep["b1_bcast"][0][None, :]
        out[c * NPC : (c + 1) * NPC] = z[:NPC]
    return out


def _build_program(prep):
    NCH, base, callsz, tot = prep["NCH"], prep["base"], prep["callsz"], prep["tot"]
    nc = bacc.Bacc(
        "TRN2",
        target_bir_lowering=False,
        debug=False,
        num_devices=M,
        num_swdge_queues=4,
        dynamic_dma_scratch_size=24576,
    )
    # I/O
    xT = nc.dram_tensor("xT", [D, NPAD], BF16, kind="ExternalInput")
    idx_io = [
        nc.dram_tensor(f"idx{s}", [128, int(tot[s]) * 8], I16, kind="ExternalInput")
        for s in range(2)
    ]
    s_io = [
        nc.dram_tensor(f"smat{s}", [128, int(tot[s]), 128], BF16, kind="ExternalInput")
        for s in range(2)
    ]
    fc0_w = nc.dram_tensor("fc0_w", [D, H], BF16, kind="ExternalInput")
    b0col = nc.dram_tensor("b0col", [H, 1], F32, kind="ExternalInput")
    wtop = nc.dram_tensor("wtop", [L, H, K * H], BF16, kind="ExternalInput")
    wbot = nc.dram_tensor("wbot", [L, H, K * H], BF16, kind="ExternalInput")
    env_w = nc.dram_tensor("env_w", [L, H, K], BF16, kind="ExternalInput")
    expb = nc.dram_tensor("expb", [L, 128, K], F32, kind="ExternalInput")
    fc1_w = nc.dram_tensor("fc1_w", [H, C], BF16, kind="ExternalInput")
    b1 = nc.dram_tensor("b1", [128, C], F32, kind="ExternalInput")
    out_io = nc.dram_tensor("out", [NPAD, C], F32, kind="ExternalOutput")

    # internal DRAM: split gather tables + per-layer AllGather inputs
    tlo = [
        nc.dram_tensor(f"tlo{l}", [LO, H], BF16, kind="Internal", addr_space="Shared")
        for l in range(L)
    ]
    thi = [
        nc.dram_tensor(f"thi{l}", [HI, H], BF16, kind="Internal", addr_space="Shared")
        for l in range(L)
    ]
    agin_a = [nc.dram_tensor(f"agina{l}", [128, TSPL, H], BF16, kind="Internal") for l in range(L)]
    agin_b = [nc.dram_tensor(f"aginb{l}", [128, T - TSPL, H], BF16, kind="Internal") for l in range(L)]

    RG = [list(range(M))]

    def ag_a(l):
        nc.gpsimd.collective_compute(
            "AllGather", mybir.AluOpType.bypass, replica_groups=RG,
            ins=[agin_a[l][:]], outs=[tlo[l][:]],
        )

    def ag_b(l):
        nc.gpsimd.collective_compute(
            "AllGather", mybir.AluOpType.bypass, replica_groups=RG,
            ins=[agin_b[l][:]], outs=[thi[l][:]],
        )

    with tile.TileContext(nc) as tc:
        with (
            tc.tile_pool(name="const", bufs=1) as const,
            tc.tile_pool(name="gsb", bufs=3) as gp,
            tc.tile_pool(name="ssb", bufs=2) as sp,
            tc.tile_pool(name="wsb", bufs=4) as sb,
            tc.tile_pool(name="ps1", bufs=1, space="PSUM") as ps1,
            tc.tile_pool(name="ps2", bufs=2, space="PSUM") as ps2,
            tc.tile_pool(name="pst", bufs=2, space="PSUM") as pst,
        ):
            ident = const.tile([128, 128], F32)
            make_identity(nc, ident[:])
            ident_bf = const.tile([128, 128], BF16)
            nc.vector.tensor_copy(ident_bf[:], ident[:])
            fc0w_sb = const.tile([D, H], BF16)
            nc.sync.dma_start(fc0w_sb[:], fc0_w[:])
            b0_sb = const.tile([H, 1], F32)
            nc.sync.dma_start(b0_sb[:], b0col[:])
            wtop_sb = [const.tile([H, K * H], BF16, tag=f"wtop{l}", name=f"wtop{l}") for l in range(L)]
            wbot_sb = [const.tile([H, K * H], BF16, tag=f"wbot{l}", name=f"wbot{l}") for l in range(L)]
            envw_sb = [const.tile([H, K], BF16, tag=f"envw{l}", name=f"envw{l}") for l in range(L)]
            expb_sb = [const.tile([128, K], F32, tag=f"expb{l}", name=f"expb{l}") for l in range(L)]
            for l in range(L):
                nc.sync.dma_start(wtop_sb[l][:], wtop[l])
                nc.sync.dma_start(wbot_sb[l][:], wbot[l])
                nc.sync.dma_start(envw_sb[l][:], env_w[l])
                nc.sync.dma_start(expb_sb[l][:], expb[l])
            fc1w_sb = const.tile([H, C], BF16)
            nc.sync.dma_start(fc1w_sb[:], fc1_w[:])
            b1_sb = const.tile([128, C], F32)
            nc.sync.dma_start(b1_sb[:], b1[:])
            idx_sb = [
                const.tile([128, int(tot[s]) * 8], I16, tag=f"idx{s}", name=f"idxsb{s}") for s in range(2)
            ]
            for s in range(2):
                nc.sync.dma_start(idx_sb[s][:], idx_io[s][:])
            hT_bf = const.tile([H, NPAD], BF16)  # feat-major h (matmul operand)
            h_node = const.tile([128, T, 128], BF16)  # node-major h blocks

            # ---------------- fc0 ----------------
            for tb in range(0, T, 4):
                nt = min(4, T - tb)
                xt = sb.tile([D, 512], BF16, tag="xt")
                nc.sync.dma_start(
                    xt[:, : nt * 128], xT[:, tb * 128 : (tb + nt) * 128]
                )
                for ti in range(nt):
                    t = tb + ti
                    z = ps2.tile([H, 128], F32, tag="gcn")
                    nc.tensor.matmul(
                        z[:], fc0w_sb[:], xt[:, ti * 128 : (ti + 1) * 128],
                        start=True, stop=True,
                    )
                    h0t = sb.tile([H, 128], BF16, tag="h0t")
                    nc.scalar.activation(
                        h0t[:], z[:], mybir.ActivationFunctionType.Relu, bias=b0_sb[:, 0:1]
                    )
                    nc.vector.tensor_copy(hT_bf[:, t * 128 : (t + 1) * 128], h0t[:])
                    ztr = pst.tile([128, H], BF16, tag="trb")
                    nc.tensor.transpose(ztr[:], h0t[:], ident_bf[:])
                    hnb = h_node[:, t, :]
                    nc.vector.tensor_copy(hnb, ztr[:])
                    if t == TSPL - 1:
                        nc.sync.dma_start(agin_a[0][:], h_node[:, 0:TSPL, :])
                        ag_a(0)
                    elif t == T - 1:
                        nc.sync.dma_start(agin_b[0][:], h_node[:, TSPL:T, :])
            ag_b(0)

            # ---------------- conv layers ----------------
            for l in range(L):
                last = l == L - 1
                if True:
                    # gather calls: lo on queues {0,2}, hi on queues {1,3};
                    # alternating queues give each ring 3 call-windows to
                    # drain before its next desc-gen needs the space back.
                    gtiles = [[], []]
                    off = [0, 0]
                    for g in range(max(len(callsz[0]), len(callsz[1]))):
                        for s in range(2):
                            if g < len(callsz[s]):
                                cs = callsz[s][g]
                                gt = gp.tile([128, CALL, H], BF16, tag=f"G{s}")
                                src = tlo[l] if s == 0 else thi[l]
                                nc.gpsimd.dma_gather(
                                    gt[:, :cs, :],
                                    src[:, :],
                                    idx_sb[s][:, off[s] : off[s] + cs * 8],
                                    num_idxs=cs * 128,
                                    num_idxs_reg=cs * 128,
                                    elem_size=H,
                                    single_packet=False,
                                    queue_num=2 * s + (g & 1),
                                )
                                gtiles[s].append(gt)
                                off[s] += cs * 8
                    # streamed S-matrix windows (SWIN chunks each)
                    stiles = [[], []]
                    for g in range(max(-(-int(tot[0]) // SWIN), -(-int(tot[1]) // SWIN))):
                        for s in range(2):
                            c0 = g * SWIN
                            if c0 < int(tot[s]):
                                cs = min(SWIN, int(tot[s]) - c0)
                                st = sp.tile([128, SWIN, 128], BF16, tag=f"S{s}")
                                nc.sync.dma_start(
                                    st[:, :cs, :], s_io[s][:, c0 : c0 + cs, :]
                                )
                                stiles[s].append(st)

                    # expert-gate softmax for every tile depends only on the
                    # previous layer's hT_bf: compute it all up front so it
                    # rides the gather shadow instead of stalling the dense
                    # pipeline's DVE FIFO mid-layer.
                    e_all = sb.tile([128, T * K], F32, tag="eall")
                    for t in range(T):
                        hsl = hT_bf[:, t * 128 : (t + 1) * 128]
                        pe = ps1.tile([128, K], F32, tag="e")
                        nc.tensor.matmul(pe[:], hsl, envw_sb[l][:], start=True, stop=True)
                        ea = e_all[:, t * K : (t + 1) * K]
                        nc.scalar.activation(
                            ea, pe[:], mybir.ActivationFunctionType.Exp
                        )
                        nc.vector.tensor_mul(ea, ea, expb_sb[l][:])
                        esum = sb.tile([128, 1], F32, tag="esum")
                        nc.vector.reduce_sum(esum[:], ea, axis=mybir.AxisListType.X)
                        nc.vector.reciprocal(esum[:], esum[:])
                        nc.vector.tensor_scalar_mul(ea, ea, esum[:, 0:1])

                    for t in range(T):
                        chunks = []
                        for s in range(2):
                            for j in range(NCH[t, s]):
                                chunks.append((s, int(base[t, s]) + j))
                        pg = ps2.tile([H, 128], F32, tag="gcn")
                        for j, (s, ch) in enumerate(chunks):
                            gt = gtiles[s][ch // CALL]
                            st = stiles[s][ch // SWIN]
                            nc.tensor.matmul(
                                pg[:],
                                gt[:, ch % CALL, :],
                                st[:, ch % SWIN, :],
                                start=(j == 0),
                                stop=(j == len(chunks) - 1),
                            )
                        gcn_bf = sb.tile([H, 128], BF16, tag="gcnbf")
                        nc.scalar.activation(
                            gcn_bf[:], pg[:], mybir.ActivationFunctionType.Copy
                        )

                        hsl = hT_bf[:, t * 128 : (t + 1) * 128]
                        po = ps2.tile([128, K * H], F32, tag="O")
                        nc.tensor.matmul(
                            po[:], gcn_bf[:], wtop_sb[l][:], start=True, stop=False
                        )
                        nc.tensor.matmul(
                            po[:], hsl, wbot_sb[l][:], start=False, stop=True
                        )
                        e_sb = e_all[:, t * K : (t + 1) * K]

                        mixs = [sb.tile([128, H], F32, tag=f"mix{i}", name=f"mix{i}") for i in range(4)]
                        for k in range(K):
                            nc.scalar.activation(
                                mixs[k][:],
                                po[:, k * H : (k + 1) * H],
                                mybir.ActivationFunctionType.Copy,
                                scale=e_sb[:, k : k + 1],
                            )
                        nc.vector.tensor_add(mixs[0][:], mixs[0][:], mixs[1][:])
                        nc.vector.tensor_add(mixs[2][:], mixs[2][:], mixs[3][:])
                        nc.vector.tensor_add(mixs[0][:], mixs[0][:], mixs[2][:])
                        hn = h_node[:, t, :]
                        nc.vector.tensor_add(mixs[0][:], mixs[0][:], hn)
                        nc.scalar.activation(
                            hn, mixs[0][:], mybir.ActivationFunctionType.Relu
                        )
                        ptr = pst.tile([128, H], BF16, tag="trb")
                        nc.tensor.transpose(ptr[:], hn, ident_bf[:])
                        if not last:
                            nc.vector.tensor_copy(
                                hT_bf[:, t * 128 : (t + 1) * 128], ptr[:]
                            )
                            if t == TSPL - 1:
                                nc.sync.dma_start(
                                    agin_a[1][:], h_node[:, 0:TSPL, :]
                                )
                                ag_a(1)
                            elif t == T - 1:
                                nc.sync.dma_start(
                                    agin_b[1][:], h_node[:, TSPL:T, :]
                                )
                        else:
                            h2T = sb.tile([H, 128], BF16, tag="h2T")
                            nc.vector.tensor_copy(h2T[:], ptr[:])
                            pc = ps1.tile([128, C], F32, tag="c")
                            nc.tensor.matmul(
                                pc[:], h2T[:], fc1w_sb[:], start=True, stop=True
                            )
                            ob = sb.tile([128, C], F32, tag="ob")
                            nc.vector.tensor_add(ob[:], pc[:], b1_sb[:])
                            nc.sync.dma_start(
                                out_io[t * 128 : (t + 1) * 128, :], ob[:]
                            )
                if not last:
                    ag_b(1)
    nc.compile()
    return nc


def _in_maps(prep):
    maps = []
    for c in range(M):
        m = {
            "xT": prep["xT"][c],
            "fc0_w": prep["fc0_w"],
            "b0col": prep["b0"][:, None].copy(),
            "wtop": prep["wtop"],
            "wbot": prep["wbot"],
            "env_w": prep["env_w_bf"],
            "expb": prep["expb"].astype(np.float32),
            "fc1_w": prep["fc1_w"],
            "b1": prep["b1_bcast"],
        }
        for s in range(2):
            m[f"idx{s}"] = prep["idx_arr"][c, s]
            m[f"smat{s}"] = prep["s_arr"][c, s]
        maps.append(m)
    return maps


_compiled = {}


def _get_compiled(prep, key):
    if key not in _compiled:
        _compiled[key] = _build_program(prep)
    return _compiled[key]


def kernel(trace=False, **inputs):
    inputs = {k: np.asarray(v) for k, v in inputs.items()}
    prep = _preprocess(**inputs)
    key = hash(inputs["edge_index"].tobytes()) ^ hash(inputs["x"].tobytes()[:4096])
    nc = _get_compiled(prep, key)
    res = bass_utils.run_bass_kernel_spmd(
        nc, _in_maps(prep), core_ids=list(range(M)), trace=trace
    )
    out = np.zeros((N, C), np.float32)
    for c in range(M):
        out[c * NPC : (c + 1) * NPC] = res.results[c]["out"][:NPC]
    kernel.last_exec_time_ns = res.exec_time_ns
    kernel.last_results = res
    return out


# revision 31
# speedup vs baseline: 1.0439x; 1.0439x over previous
"""CaNet (moe_routing GNN) forward on 8 Trainium2 NeuronCores.

Sharding: nodes are range-partitioned across the 8 cores (6250 each, padded
to 6272 = 49*128). Each core owns the edges whose *destination* lands in its
node range. The GCN aggregation out[col] += val * h[row] is computed as a
sequence of tiny one-hot matmuls on the TensorEngine:

  - edges are sorted by destination tile (groups of 128 dest nodes) on the
    host and padded to multiples of 128 ("chunks");
  - the source features h[row] for one chunk are fetched from a replicated
    node-major bf16 table in HBM with the GpSimd dma_gather custom op.
    The table is split by *slab half*: rows [0,3200) of every core's padded
    slab form the "lo" table (8*3200=25600 rows), rows [3200,6272) the "hi"
    table (8*3072=24576 rows); both halves are int16-indexable and every
    chunk draws from a single half ("lo"/"hi" streams);
  - lo gathers run on SWDGE queue 0 and hi gathers on queue 1 so one call's
    Q7 descriptor generation overlaps the other call's HBM drain (a shared
    ring otherwise serializes gen -> drain -> gen);
  - a [128e x 128d] selection matrix S (S[e,d] = (d == ldest[e]) * val[e])
    is prebuilt on the host per chunk and streamed from HBM in 32-chunk
    windows (building S on the DVE per chunk serializes the vector engine
    behind gather-gated matmuls);
  - psum_gcnT[f,d] += G_chunk.T @ S accumulates over the tile's chunks.

Dense per-node work (expert gate softmax, the K=4 expert convs, mixing,
residual relu, fc0/fc1) runs in bf16 matmuls + f32 psum per 128-node tile.
Between layers the updated h slab is AllGather'd in two pieces: the first
25 tiles' rows (the lo half) are gathered as soon as they are computed, so
the next layer's lo gather calls start while the hi half is still being
computed/gathered.

The per-core *program* is identical (SPMD); all per-core variation (gather
indices, selection metadata, x slab) arrives via ExternalInputs. Chunk
counts per (tile, half) are max'd across cores so the schedule is static;
padding slots use idx=0 / val=0 and contribute zero. Gather calls are 64
chunks (8192 indices) except a short final call per stream, so there is no
call-level index padding.
"""

import sys

sys.path.insert(0, "/opt/trn_rl_repo")

import numpy as np
import ml_dtypes

import concourse.bacc as bacc
import concourse.tile as tile
import concourse.mybir as mybir
import concourse.bass as bass
from concourse import bass_utils
from concourse.masks import make_identity

# Problem constants (hardcoded per contract).
N = 50000
E = 800000
D = 128  # input dim
H = 128  # hidden dim
C = 47  # classes
K = 4  # experts
L = 2  # conv layers
M = 8  # cores

NPC = N // M  # 6250 nodes per core
T = (NPC + 127) // 128  # 49 tiles per core
NPAD = T * 128  # 6272
TSPL = 25  # tiles in the "lo" slab half
SPL = TSPL * 128  # 3200 rows: slab rows [0,SPL) are "lo", rest "hi"
SPH = NPAD - SPL  # 3072
LO = M * SPL  # 25600 lo-table rows (int16-safe)
HI = M * SPH  # 24576 hi-table rows
CALL = 32  # chunks per full dma_gather call (4096 indices)
SWIN = 32  # chunks per streamed S-matrix window (aligned with gather calls)

F32 = mybir.dt.float32
BF16 = mybir.dt.bfloat16
I16 = mybir.dt.int16
BF = ml_dtypes.bfloat16


def _preprocess(x, edge_index, fc0_w, fc0_b, fc1_w, fc1_b, env_w, env_b, conv_w):
    """Host-side: degree/value computation, edge sort, static chunk schedule,
    per-core gather/selection arrays, weight packing."""
    row = np.asarray(edge_index[0], np.int64)
    col = np.asarray(edge_index[1], np.int64)

    deg = np.bincount(col, minlength=N).astype(np.float32)
    dinv = np.where(deg > 0, 1.0 / np.sqrt(deg), 0.0).astype(np.float32)
    val = (dinv[col] * dinv[row]).astype(np.float32)

    core = col // NPC
    dloc = col % NPC
    tl = dloc // 128
    ld = (dloc % 128).astype(np.float32)
    score = row // NPC  # source core
    soff = row % NPC  # offset within source slab (< NPC <= NPAD)
    half = (soff >= SPL).astype(np.int64)
    # agin buffers are shipped partition-major ([128p, tiles, H]), so table
    # row for node offset off = t*128+p is p*TILES + t within the core block.
    tA = soff // 128
    pA = soff % 128
    tB = (soff - SPL) // 128
    idx16 = np.where(
        half == 1,
        score * SPH + pA * (T - TSPL) + tB,
        score * SPL + pA * TSPL + tA,
    )

    # group = (core, half, tile); count chunks per (tile, half) max'd over cores
    cnt = np.bincount((core * T + tl) * 2 + half, minlength=M * T * 2).reshape(
        M, T, 2
    )
    nch = -(-cnt // 128)  # ceil div, [M, T, 2]
    NCH = nch.max(axis=0)  # [T, 2] static schedule
    assert NCH.sum() > 0
    tot = NCH.sum(axis=0)  # [2] total chunks per stream (no call padding)
    # call sizes: full CALL-chunk calls plus one short final call
    callsz = []
    for s in range(2):
        t_s = int(tot[s])
        cs = [CALL] * (t_s // CALL)
        if t_s % CALL:
            cs.append(t_s % CALL)
        callsz.append(cs)

    # stream-local chunk base per tile (exclusive cumsum), shared across cores
    base = np.zeros((T, 2), np.int64)
    base[1:] = NCH[:-1].cumsum(axis=0)

    # within-group rank for every edge
    gkey = (core * 2 + half) * T + tl
    order = np.argsort(gkey, kind="stable")
    gsort = gkey[order]
    starts = np.searchsorted(gsort, np.arange(M * 2 * T))
    rank = np.arange(E, dtype=np.int64) - starts[gsort]
    # slot within (core, half) stream
    slot = np.empty(E, np.int64)
    slot[order] = base[tl[order], half[order]] * 128 + rank

    idx_arr = np.zeros((M, 2), object)
    ld_arr = np.zeros((M, 2), object)
    v_arr = np.zeros((M, 2), object)
    s_arr = np.zeros((M, 2), object)
    lanes = np.arange(128)
    for c in range(M):
        for s in range(2):
            nslot = int(tot[s]) * 128
            ia = np.zeros(nslot, np.int16)
            la = np.zeros(nslot, np.int64)
            va = np.zeros(nslot, np.float32)
            sel = (core == c) & (half == s)
            ia[slot[sel]] = idx16[sel].astype(np.int16)
            la[slot[sel]] = ld[sel].astype(np.int64)
            va[slot[sel]] = val[sel]
            # wrap indices for dma_gather: per call [16, C*8] tiled x8 -> [128, C*8]
            pieces = []
            off = 0
            for cs in callsz[s]:
                ni = cs * 128
                iw = ia[off : off + ni].reshape(ni // 16, 16).T  # [16, cs*8]
                pieces.append(np.tile(iw, (8, 1)))  # [128, cs*8]
                off += ni
            idx_arr[c, s] = np.concatenate(pieces, axis=1)
            ld_arr[c, s] = la.reshape(int(tot[s]), 128).T.astype(np.float32)
            v_arr[c, s] = va.reshape(int(tot[s]), 128).T.copy()
            # prebuilt selection matrices: S[e, ch, d] = (d == ld)*val
            sf = np.zeros((nslot, 128), BF)
            sf[np.arange(nslot), la] = va.astype(BF)
            sf[va == 0.0] = 0  # padding slots contribute nothing even if ld=0
            s_arr[c, s] = np.ascontiguousarray(
                sf.reshape(int(tot[s]), 128, 128).transpose(1, 0, 2)
            )

    # x slabs, transposed + padded: [128 d, NPAD n] bf16 per core
    x = np.asarray(x, np.float32)
    xT = np.zeros((M, D, NPAD), BF)
    for c in range(M):
        xT[c, :, :NPC] = x[c * NPC : (c + 1) * NPC].T.astype(BF)

    # packed weights
    conv_w = np.asarray(conv_w, np.float32)
    wtop = np.zeros((L, H, K * H), BF)
    wbot = np.zeros((L, H, K * H), BF)
    for l in range(L):
        for k in range(K):
            wtop[l, :, k * H : (k + 1) * H] = conv_w[l, k, :H].astype(BF)
            wbot[l, :, k * H : (k + 1) * H] = conv_w[l, k, H:].astype(BF)
    env_w = np.asarray(env_w, np.float32)
    env_b = np.asarray(env_b, np.float32)
    prep = dict(
        NCH=NCH,
        base=base,
        callsz=callsz,
        tot=tot,
        idx_arr=idx_arr,
        ld_arr=ld_arr,
        v_arr=v_arr,
        s_arr=s_arr,
        xT=xT,
        fc0_w=np.asarray(fc0_w, BF),
        b0=np.asarray(fc0_b, np.float32),
        wtop=wtop,
        wbot=wbot,
        env_w_bf=env_w.astype(BF),
        expb=np.stack([np.tile(np.exp(env_b[l]), (128, 1)) for l in range(L)]),
        fc1_w=np.asarray(fc1_w, BF),
        b1_bcast=np.tile(np.asarray(fc1_b, np.float32), (128, 1)),
    )
    return prep


def _emulate(prep):
    """Numpy mirror of the device program (validates schedule/indexing)."""
    NCH, base, callsz, tot = prep["NCH"], prep["base"], prep["callsz"], prep["tot"]
    h_node = np.zeros((M, NPAD, H), np.float32)
    for c in range(M):
        z = prep["xT"][c].T.astype(np.float32) @ prep["fc0_w"].astype(
            np.float32
        ) + prep["b0"]
        h_node[c] = np.maximum(z, 0.0)

    def tables(hn):
        # p-major within each core: row p*TILES + t = node t*128 + p
        tlo = np.concatenate([
            hn[c, :SPL].reshape(TSPL, 128, H).transpose(1, 0, 2).reshape(SPL, H).astype(BF)
            for c in range(M)
        ])
        thi = np.concatenate([
            hn[c, SPL:].reshape(T - TSPL, 128, H).transpose(1, 0, 2).reshape(SPH, H).astype(BF)
            for c in range(M)
        ])
        return tlo, thi

    tlo, thi = tables(h_node)

    for l in range(L):
        new_h = np.zeros_like(h_node)
        for c in range(M):
            G = [None, None]
            for s in range(2):
                ia = prep["idx_arr"][c, s]
                idxs = []
                off = 0
                for cs in callsz[s]:
                    blkw = ia[:16, off : off + cs * 8]  # [16, cs*8]
                    idxs.append(blkw.T.reshape(-1))
                    off += cs * 8
                idxs = np.concatenate(idxs).astype(np.int64)
                tab = tlo if s == 0 else thi
                G[s] = tab[idxs].astype(np.float32)
            gcnT = np.zeros((T, H, 128), np.float32)
            for t in range(T):
                acc = np.zeros((H, 128), np.float32)
                for s in range(2):
                    for jc in range(NCH[t, s]):
                        ch = base[t, s] + jc
                        g = G[s][ch * 128 : (ch + 1) * 128]
                        S = prep["s_arr"][c, s][:, ch, :].astype(np.float32)
                        acc += g.astype(BF).astype(np.float32).T @ S
                gcnT[t] = acc
            hT_bf = h_node[c].T.astype(BF)
            for t in range(T):
                sl = slice(t * 128, (t + 1) * 128)
                ht = hT_bf[:, sl].astype(np.float32)
                z = ht.T @ prep["env_w_bf"][l].astype(np.float32)
                e = np.exp(z) * prep["expb"][l][0][None, :]
                e = e / e.sum(axis=1, keepdims=True)
                gt = gcnT[t].astype(BF).astype(np.float32)
                O = gt.T @ prep["wtop"][l].astype(np.float32) + ht.T @ prep[
                    "wbot"
                ][l].astype(np.float32)
                O = O.reshape(128, K, H)
                mixed = np.einsum("nk,nkh->nh", e, O)
                new_h[c, sl] = np.maximum(mixed + h_node[c, sl], 0.0)
        h_node = new_h
        tlo, thi = tables(h_node)

    out = np.zeros((N, C), np.float32)
    for c in range(M):
        z = h_node[c] @ prep["fc1_w"].astype(np.float32) + prep["b1_bcast"][0][None, :]
        out[c * NPC : (c + 1) * NPC] = z[:NPC]
    return out


def _build_program(prep):
    NCH, base, callsz, tot = prep["NCH"], prep["base"], prep["callsz"], prep["tot"]
    nc = bacc.Bacc(
        "TRN2",
        target_bir_lowering=False,
        debug=False,
        num_devices=M,
        num_swdge_queues=4,
        dynamic_dma_scratch_size=24576,
    )
    # I/O
    xT = nc.dram_tensor("xT", [D, NPAD], BF16, kind="ExternalInput")
    idx_io = [
        nc.dram_tensor(f"idx{s}", [128, int(tot[s]) * 8], I16, kind="ExternalInput")
        for s in range(2)
    ]
    s_io = [
        nc.dram_tensor(f"smat{s}", [128, int(tot[s]), 128], BF16, kind="ExternalInput")
        for s in range(2)
    ]
    fc0_w = nc.dram_tensor("fc0_w", [D, H], BF16, kind="ExternalInput")
    b0col = nc.dram_tensor("b0col", [H, 1], F32, kind="ExternalInput")
    wtop = nc.dram_tensor("wtop", [L, H, K * H], BF16, kind="ExternalInput")
    wbot = nc.dram_tensor("wbot", [L, H, K * H], BF16, kind="ExternalInput")
    env_w = nc.dram_tensor("env_w", [L, H, K], BF16, kind="ExternalInput")
    expb = nc.dram_tensor("expb", [L, 128, K], F32, kind="ExternalInput")
    fc1_w = nc.dram_tensor("fc1_w", [H, C], BF16, kind="ExternalInput")
    b1 = nc.dram_tensor("b1", [128, C], F32, kind="ExternalInput")
    out_io = nc.dram_tensor("out", [NPAD, C], F32, kind="ExternalOutput")

    # internal DRAM: split gather tables + per-layer AllGather inputs
    tlo = [
        nc.dram_tensor(f"tlo{l}", [LO, H], BF16, kind="Internal", addr_space="Shared")
        for l in range(L)
    ]
    thi = [
        nc.dram_tensor(f"thi{l}", [HI, H], BF16, kind="Internal", addr_space="Shared")
        for l in range(L)
    ]
    agin_a = [nc.dram_tensor(f"agina{l}", [128, TSPL, H], BF16, kind="Internal") for l in range(L)]
    agin_b = [nc.dram_tensor(f"aginb{l}", [128, T - TSPL, H], BF16, kind="Internal") for l in range(L)]

    RG = [list(range(M))]

    def ag_a(l):
        nc.gpsimd.collective_compute(
            "AllGather", mybir.AluOpType.bypass, replica_groups=RG,
            ins=[agin_a[l][:]], outs=[tlo[l][:]],
        )

    def ag_b(l):
        nc.gpsimd.collective_compute(
            "AllGather", mybir.AluOpType.bypass, replica_groups=RG,
            ins=[agin_b[l][:]], outs=[thi[l][:]],
        )

    with tile.TileContext(nc) as tc:
        with (
            tc.tile_pool(name="const", bufs=1) as const,
            tc.tile_pool(name="gsb", bufs=4) as gp,
            tc.tile_pool(name="ssb", bufs=2) as sp,
            tc.tile_pool(name="wsb", bufs=4) as sb,
            tc.tile_pool(name="ps1", bufs=1, space="PSUM") as ps1,
            tc.tile_pool(name="ps2", bufs=2, space="PSUM") as ps2,
            tc.tile_pool(name="pst", bufs=2, space="PSUM") as pst,
        ):
            ident = const.tile([128, 128], F32)
            make_identity(nc, ident[:])
            ident_bf = const.tile([128, 128], BF16)
            nc.vector.tensor_copy(ident_bf[:], ident[:])
            fc0w_sb = const.tile([D, H], BF16)
            nc.sync.dma_start(fc0w_sb[:], fc0_w[:])
            b0_sb = const.tile([H, 1], F32)
            nc.sync.dma_start(b0_sb[:], b0col[:])
            wtop_sb = [const.tile([H, K * H], BF16, tag=f"wtop{l}", name=f"wtop{l}") for l in range(L)]
            wbot_sb = [const.tile([H, K * H], BF16, tag=f"wbot{l}", name=f"wbot{l}") for l in range(L)]
            envw_sb = [const.tile([H, K], BF16, tag=f"envw{l}", name=f"envw{l}") for l in range(L)]
            expb_sb = [const.tile([128, K], F32, tag=f"expb{l}", name=f"expb{l}") for l in range(L)]
            for l in range(L):
                nc.sync.dma_start(wtop_sb[l][:], wtop[l])
                nc.sync.dma_start(wbot_sb[l][:], wbot[l])
                nc.sync.dma_start(envw_sb[l][:], env_w[l])
                nc.sync.dma_start(expb_sb[l][:], expb[l])
            fc1w_sb = const.tile([H, C], BF16)
            nc.sync.dma_start(fc1w_sb[:], fc1_w[:])
            b1_sb = const.tile([128, C], F32)
            nc.sync.dma_start(b1_sb[:], b1[:])
            idx_sb = [
                const.tile([128, int(tot[s]) * 8], I16, tag=f"idx{s}", name=f"idxsb{s}") for s in range(2)
            ]
            for s in range(2):
                nc.sync.dma_start(idx_sb[s][:], idx_io[s][:])
            hT_bf = const.tile([H, NPAD], BF16)  # feat-major h (matmul operand)
            h_node = const.tile([128, T, 128], BF16)  # node-major h blocks

            # ---------------- fc0 ----------------
            for tb in range(0, T, 4):
                nt = min(4, T - tb)
                xt = sb.tile([D, 512], BF16, tag="xt")
                nc.sync.dma_start(
                    xt[:, : nt * 128], xT[:, tb * 128 : (tb + nt) * 128]
                )
                for ti in range(nt):
                    t = tb + ti
                    z = ps2.tile([H, 128], F32, tag="gcn")
                    nc.tensor.matmul(
                        z[:], fc0w_sb[:], xt[:, ti * 128 : (ti + 1) * 128],
                        start=True, stop=True,
                    )
                    h0t = sb.tile([H, 128], BF16, tag="h0t")
                    nc.scalar.activation(
                        h0t[:], z[:], mybir.ActivationFunctionType.Relu, bias=b0_sb[:, 0:1]
                    )
                    nc.vector.tensor_copy(hT_bf[:, t * 128 : (t + 1) * 128], h0t[:])
                    ztr = pst.tile([128, H], BF16, tag="trb")
                    nc.tensor.transpose(ztr[:], h0t[:], ident_bf[:])
                    hnb = h_node[:, t, :]
                    nc.vector.tensor_copy(hnb, ztr[:])
                    if t == TSPL - 1:
                        nc.sync.dma_start(agin_a[0][:], h_node[:, 0:TSPL, :])
                        ag_a(0)
                    elif t == T - 1:
                        nc.sync.dma_start(agin_b[0][:], h_node[:, TSPL:T, :])
            ag_b(0)

            # ---------------- conv layers ----------------
            for l in range(L):
                last = l == L - 1
                if True:
                    # gather calls: lo on queues {0,2}, hi on queues {1,3};
                    # alternating queues give each ring 3 call-windows to
                    # drain before its next desc-gen needs the space back.
                    gtiles = [[], []]
                    off = [0, 0]
                    for g in range(max(len(callsz[0]), len(callsz[1]))):
                        for s in range(2):
                            if g < len(callsz[s]):
                                cs = callsz[s][g]
                                gt = gp.tile([128, CALL, H], BF16, tag=f"G{s}")
                                src = tlo[l] if s == 0 else thi[l]
                                nc.gpsimd.dma_gather(
                                    gt[:, :cs, :],
                                    src[:, :],
                                    idx_sb[s][:, off[s] : off[s] + cs * 8],
                                    num_idxs=cs * 128,
                                    num_idxs_reg=cs * 128,
                                    elem_size=H,
                                    single_packet=False,
                                    queue_num=2 * s + (g & 1),
                                )
                                gtiles[s].append(gt)
                                off[s] += cs * 8
                    # streamed S-matrix windows (SWIN chunks each)
                    stiles = [[], []]
                    for g in range(max(-(-int(tot[0]) // SWIN), -(-int(tot[1]) // SWIN))):
                        for s in range(2):
                            c0 = g * SWIN
                            if c0 < int(tot[s]):
                                cs = min(SWIN, int(tot[s]) - c0)
                                st = sp.tile([128, SWIN, 128], BF16, tag=f"S{s}")
                                nc.sync.dma_start(
                                    st[:, :cs, :], s_io[s][:, c0 : c0 + cs, :]
                                )
                                stiles[s].append(st)

                    # expert-gate softmax for every tile depends only on the
                    # previous layer's hT_bf: compute it all up front so it
                    # rides the gather shadow instead of stalling the dense
                    # pipeline's DVE FIFO mid-layer.
                    e_all = sb.tile([128, T * K], F32, tag="eall")
                    for t in range(T):
                        hsl = hT_bf[:, t * 128 : (t + 1) * 128]
                        pe = ps1.tile([128, K], F32, tag="e")
                        nc.tensor.matmul(pe[:], hsl, envw_sb[l][:], start=True, stop=True)
                        ea = e_all[:, t * K : (t + 1) * K]
                        nc.scalar.activation(
                            ea, pe[:], mybir.ActivationFunctionType.Exp
                        )
                        nc.vector.tensor_mul(ea, ea, expb_sb[l][:])
                        esum = sb.tile([128, 1], F32, tag="esum")
                        nc.vector.reduce_sum(esum[:], ea, axis=mybir.AxisListType.X)
                        nc.vector.reciprocal(esum[:], esum[:])
                        nc.vector.tensor_scalar_mul(ea, ea, esum[:, 0:1])

                    for t in range(T):
                        chunks = []
                        for s in range(2):
                            for j in range(NCH[t, s]):
                                chunks.append((s, int(base[t, s]) + j))
                        pg = ps2.tile([H, 128], F32, tag="gcn")
                        for j, (s, ch) in enumerate(chunks):
                            gt = gtiles[s][ch // CALL]
                            st = stiles[s][ch // SWIN]
                            nc.tensor.matmul(
                                pg[:],
                                gt[:, ch % CALL, :],
                                st[:, ch % SWIN, :],
                                start=(j == 0),
                                stop=(j == len(chunks) - 1),
                            )
                        gcn_bf = sb.tile([H, 128], BF16, tag="gcnbf")
                        nc.scalar.activation(
                            gcn_bf[:], pg[:], mybir.ActivationFunctionType.Copy
                        )

                        hsl = hT_bf[:, t * 128 : (t + 1) * 128]
                        po = ps2.tile([128, K * H], F32, tag="O")
                        nc.tensor.matmul(
                            po[:], gcn_bf[:], wtop_sb[l][:], start=True, stop=False
                        )
                        nc.tensor.matmul(
                            po[:], hsl, wbot_sb[l][:], start=False, stop=True
                        )
                        e_sb = e_all[:, t * K : (t + 1) * K]

                        mixs = [sb.tile([128, H], F32, tag=f"mix{i}", name=f"mix{i}") for i in range(4)]
                        for k in range(K):
                            nc.scalar.activation(
                                mixs[k][:],
                                po[:, k * H : (k + 1) * H],
                                mybir.ActivationFunctionType.Copy,
                                scale=e_sb[:, k : k + 1],
                            )
                        nc.vector.tensor_add(mixs[0][:], mixs[0][:], mixs[1][:])
                        nc.vector.tensor_add(mixs[2][:], mixs[2][:], mixs[3][:])
                        nc.vector.tensor_add(mixs[0][:], mixs[0][:], mixs[2][:])
                        hn = h_node[:, t, :]
                        nc.vector.tensor_add(mixs[0][:], mixs[0][:], hn)
                        nc.scalar.activation(
                            hn, mixs[0][:], mybir.ActivationFunctionType.Relu
                        )
                        ptr = pst.tile([128, H], BF16, tag="trb")
                        nc.tensor.transpose(ptr[:], hn, ident_bf[:])
                        if not last:
                            nc.vector.tensor_copy(
                                hT_bf[:, t * 128 : (t + 1) * 128], ptr[:]
                            )
                            if t == TSPL - 1:
                                nc.sync.dma_start(
                                    agin_a[1][:], h_node[:, 0:TSPL, :]
                                )
                                ag_a(1)
                            elif t == T - 1:
                                nc.sync.dma_start(
                                    agin_b[1][:], h_node[:, TSPL:T, :]
                                )
                        else:
                            h2T = sb.tile([H, 128], BF16, tag="h2T")
                            nc.vector.tensor_copy(h2T[:], ptr[:])
                            pc = ps1.tile([128, C], F32, tag="c")
                            nc.tensor.matmul(
                                pc[:], h2T[:], fc1w_sb[:], start=True, stop=True
                            )
                            ob = sb.tile([128, C], F32, tag="ob")
                            nc.vector.tensor_add(ob[:], pc[:], b1_sb[:])
                            nc.sync.dma_start(
                                out_io[t * 128 : (t + 1) * 128, :], ob[:]
                            )
                if not last:
                    ag_b(1)
    nc.compile()
    return nc


def _in_maps(prep):
    maps = []
    for c in range(M):
        m = {
            "xT": prep["xT"][c],
            "fc0_w": prep["fc0_w"],
            "b0col": prep["b0"][:, None].copy(),
            "wtop": prep["wtop"],
            "wbot": prep["wbot"],
            "env_w": prep["env_w_bf"],
            "expb": prep["expb"].astype(np.float32),
            "fc1_w": prep["fc1_w"],
            "b1": prep["b1_bcast"],
        }
        for s in range(2):
            m[f"idx{s}"] = prep["idx_arr"][c, s]
            m[f"smat{s}"] = prep["s_arr"][c, s]
        maps.append(m)
    return maps


_compiled = {}


def _get_compiled(prep, key):
    if key not in _compiled:
        _compiled[key] = _build_program(prep)
    return _compiled[key]


def kernel(trace=False, **inputs):
    inputs = {k: np.asarray(v) for k, v in inputs.items()}
    prep = _preprocess(**inputs)
    key = hash(inputs["edge_index"].tobytes()) ^ hash(inputs["x"].tobytes()[:4096])
    nc = _get_compiled(prep, key)
    res = bass_utils.run_bass_kernel_spmd(
        nc, _in_maps(prep), core_ids=list(range(M)), trace=trace
    )
    out = np.zeros((N, C), np.float32)
    for c in range(M):
        out[c * NPC : (c + 1) * NPC] = res.results[c]["out"][:NPC]
    kernel.last_exec_time_ns = res.exec_time_ns
    kernel.last_results = res
    return out
